# revision 28
# baseline (speedup 1.0000x reference)
"""Trainium2 Bass kernel for nn_FLASHAttention_3650722201963.

Reference computation (per batch b, chunks g of size C=256 over SRC=4096):
    x = value[:, b, :]                      # [SRC, D]   (query/key are unused!)
    v = tanh(x @ Wve + bve)                 # [.., E]
    z = tanh(x @ Wxs + bxs)                 # [.., S]
    q_quad/k_quad/q_lin/k_lin = z * gamma_i + beta_i
    qk = q_quad @ k_quad^T (per chunk)      # [C, C]
    a  = relu(qk + rel_bias)^2
    v_quad = a @ v (per chunk)
    kv = sum_{g,c} k_lin^T v                # [S, E] global per batch
    v_lin = q_lin @ kv
    u = tanh(x @ Wue + bue)
    o = (u * (v_quad + v_lin)) @ Wod + bod  # [SRC, D]

Sharding: pure data parallel over batch (B=8) -> one batch element per core.
All matmuls in bf16 with fp32 PSUM accumulation. Host pre-transposes x to
xT [D, SRC] per core so the device never transposes activations; the weights'
natural [in, out] layout is already what the PE wants.

Two passes per core:
  pass 1: zT (kept resident), k_lin (via PE transpose), v (spilled bf16 to
          DRAM scratch), kv accumulated across all chunks in a resident PSUM
          tile.
  pass 2: qk^T -> a^T (rel_bias folds into the ACT as a per-partition bias),
          per e-tile: v_quad^T and v_lin^T accumulate into the SAME psum bank,
          u^T = tanh(.. + bue) (per-partition bias), h = u^T*(vq+vl) on DVE,
          o = h^T-matmuls accumulated over e in PSUM, bod added during the
          PSUM->SBUF copy.
"""

import numpy as np
import ml_dtypes

import concourse.bass as bass
import concourse.tile as tile
from concourse import bacc, mybir
from concourse.bass_utils import run_bass_kernel_spmd
from concourse.masks import make_identity

BF16 = mybir.dt.bfloat16
F32 = mybir.dt.float32
AF = mybir.ActivationFunctionType

D = 1024      # embed dim
E = 2048      # expanded dim
S = 128       # shrunken attn dim
C = 256       # chunk size
SRC = 4096    # sequence length
G = SRC // C  # 16 chunks
B = 8         # batch == n cores
KD = D // 128   # 8 k-tiles over D
KE = E // 128   # 16 e-tiles


def build_nc() -> bacc.Bacc:
    nc = bacc.Bacc(None, target_bir_lowering=False, debug=False)

    # ---- I/O ----
    xT_h = nc.declare_dram_parameter("xT", [128, KD, SRC], BF16, isOutput=False)
    wxs_h = nc.declare_dram_parameter("Wxs", [128, KD, S], BF16, isOutput=False)
    wve_h = nc.declare_dram_parameter("Wve", [128, KD, E], BF16, isOutput=False)
    wue_h = nc.declare_dram_parameter("Wue", [128, KD, E], BF16, isOutput=False)
    wod_h = nc.declare_dram_parameter("Wod", [128, KE, D], BF16, isOutput=False)
    bxs_h = nc.declare_dram_parameter("bxs", [128, 1], F32, isOutput=False)
    bve_h = nc.declare_dram_parameter("bve", [128, E], BF16, isOutput=False)
    bue_h = nc.declare_dram_parameter("bue", [128, KE], F32, isOutput=False)
    bod_h = nc.declare_dram_parameter("bod", [128, D], F32, isOutput=False)
    rel_h = nc.declare_dram_parameter("rel", [128, 2], F32, isOutput=False)
    gam_h = nc.declare_dram_parameter("gam", [128, 4], F32, isOutput=False)
    bet_h = nc.declare_dram_parameter("bet", [128, 4], F32, isOutput=False)
    o_h = nc.declare_dram_parameter("o", [SRC, D], F32, isOutput=True)

    v_spill = nc.dram_tensor("v_spill", [SRC, E], BF16)

    with tile.TileContext(nc) as tc:
        with (
            tc.tile_pool(name="consts", bufs=1) as consts,
            tc.tile_pool(name="xp", bufs=3) as xp,
            tc.tile_pool(name="vp", bufs=4) as vp,
            tc.tile_pool(name="small", bufs=3) as small,
            tc.tile_pool(name="op", bufs=3) as osb,
        ):
            # ---- resident constants ----
            w_xs = consts.tile([128, KD, S], BF16)
            w_ve = consts.tile([128, KD, E], BF16)
            w_ue = consts.tile([128, KD, E], BF16)
            w_od = consts.tile([128, KE, D], BF16)
            b_xs = consts.tile([128, 1], F32)
            b_ve = consts.tile([128, E], BF16)
            b_ue = consts.tile([128, KE], F32)
            b_od = consts.tile([128, D], F32)
            relb = consts.tile([128, 2], F32)
            gam = consts.tile([128, 4], F32)
            bet = consts.tile([128, 4], F32)
            ident = consts.tile([128, 128], BF16)
            zT_all = consts.tile([128, G, C], BF16)
            kv_sb = consts.tile([128, E], BF16)

            # HAM warmup + DMA-window cover: ~300 dummy matmuls (~18us) keep
            # the PE busy while the ~6MB of first-touch weights stream from
            # HBM. Without this the PE idles in 3-7us holes waiting for Wve
            # slices, re-throttling the clock to 1.2GHz (HAM) and running
            # the first ~30us of real matmuls at half rate.
            with tc.tile_pool(name="pwarm", bufs=1, space="PSUM") as pwarm:
                warm_in = consts.tile([128, 128], BF16)
                nc.vector.memset(warm_in[:], 0.0)
                warm_ps = pwarm.tile([128, 128], F32)
                for _ in range(300):
                    nc.tensor.matmul(warm_ps[:], warm_in[:], warm_in[:],
                                     start=True, stop=True)

            # Head-latency critical path: the sync HWDGE ring carries only
            # what the first chunk needs, in need-order (Wxs -> x chunk 0 ->
            # Wve streamed per k-tile). Small pass-1 consts ride the scalar
            # ring early; the 8.5MB of pass-2-only weights are emitted later
            # (inside the pass-1 loop) so DMA-semaphore recycling never makes
            # a critical load wait on a big transfer.
            nc.sync.dma_start(out=w_xs[:], in_=wxs_h[:])
            nc.scalar.dma_start(out=b_xs[:], in_=bxs_h[:])
            nc.scalar.dma_start(out=gam[:], in_=gam_h[:])
            nc.scalar.dma_start(out=bet[:], in_=bet_h[:])
            nc.scalar.dma_start(out=b_ve[:], in_=bve_h[:])
            make_identity(nc, ident[:])

            # ================= PASS 1 =================
            with (
                tc.tile_pool(name="psz", bufs=2, space="PSUM") as psz,
                tc.tile_pool(name="psv", bufs=2, space="PSUM") as psv,
                tc.tile_pool(name="pkv", bufs=1, space="PSUM") as pkv,
            ):
                kv_ps = pkv.tile([128, E], F32)

                xgs: dict = {}

                def get_xg(g):
                    if g not in xgs:
                        t = xp.tile([128, KD, C], BF16, tag="xg",
                                    name=f"xg{g}")
                        sl = slice(g * C, (g + 1) * C)
                        nc.sync.dma_start(out=t[:], in_=xT_h[:, :, sl])
                        xgs[g] = t
                    return xgs[g]

                def z_front(g):
                    """zT[g] = tanh(Wxs^T x^T + bxs) + k_lin^T affine.
                    Emitted one chunk ahead of its v-block so the PE->ACT
                    round trip hides under the previous chunk's matmuls."""
                    xg = get_xg(g)
                    z_ps = psz.tile([128, C], F32, tag="zps", name=f"zps{g}")
                    for kt in range(KD):
                        nc.tensor.matmul(
                            z_ps[:], w_xs[:, kt, :], xg[:, kt, :],
                            start=(kt == 0), stop=(kt == KD - 1),
                        )
                    nc.scalar.activation(
                        zT_all[:, g, :], z_ps[:], AF.Tanh, bias=b_xs[:, 0:1],
                    )
                    klT = small.tile([128, C], BF16, tag="klT",
                                     name=f"klT{g}")
                    nc.scalar.activation(
                        klT[:], zT_all[:, g, :], AF.Identity,
                        bias=bet[:, 3:4], scale=gam[:, 3:4],
                    )
                    return klT

                def z_back(g, klT):
                    """PE-transpose k_lin^T -> k_lin [C, S]; emitted after the
                    previous chunk's matmul block so the PE never waits on the
                    ACT affine."""
                    kl = small.tile([128, 2, S], BF16, tag="kl",
                                    name=f"kl{g}")
                    for ct in range(2):
                        t_ps = psz.tile([128, 128], BF16, tag="zps",
                                        name=f"tps{g}_{ct}")
                        nc.tensor.transpose(
                            t_ps[:], klT[:, ct * 128:(ct + 1) * 128], ident[:]
                        )
                        nc.vector.tensor_copy(kl[:, ct, :], t_ps[:])
                    return kl

                get_xg(0)
                # ns-major streaming matches the v-loop's consumption order:
                # the (ct0, ns0) psum group needs all k-tiles of e-cols 0:512.
                # xg1 is prefetched after the first slice so chunk 1's zT
                # isn't stuck behind the remaining 3MB of Wve.
                for ns in range(E // 512):
                    esl = slice(ns * 512, (ns + 1) * 512)
                    nc.sync.dma_start(out=w_ve[:, :, esl], in_=wve_h[:, :, esl])
                    if ns == 0:
                        get_xg(1)
                kls = {0: z_back(0, z_front(0))}
                klTs: dict = {}
                for g in range(G):
                    xg = get_xg(g)
                    if g + 1 < G:
                        klTs[g + 1] = z_front(g + 1)
                    if g == 4:
                        # pass-2-only weights: late enough not to stall the
                        # head, early enough to land well before pass 2
                        nc.scalar.dma_start(out=w_ue[:], in_=wue_h[:])
                        nc.scalar.dma_start(out=w_od[:], in_=wod_h[:])
                        nc.scalar.dma_start(out=b_ue[:], in_=bue_h[:])
                        nc.scalar.dma_start(out=b_od[:], in_=bod_h[:])
                        nc.scalar.dma_start(out=relb[:], in_=rel_h[:])
                    kl = kls.pop(g)

                    # v = tanh(x @ Wve + bve), spill to DRAM; kv += k_lin^T v
                    for ct in range(2):
                        vt = vp.tile([128, E], BF16, tag="v")
                        for ns in range(E // 512):
                            v_ps = psv.tile([128, 512], F32, tag="vps")
                            esl = slice(ns * 512, (ns + 1) * 512)
                            for kt in range(KD):
                                nc.tensor.matmul(
                                    v_ps[:], xg[:, kt, ct * 128:(ct + 1) * 128],
                                    w_ve[:, kt, esl],
                                    start=(kt == 0), stop=(kt == KD - 1),
                                )
                            nc.vector.tensor_add(v_ps[:], v_ps[:], b_ve[:, esl])
                            nc.scalar.activation(vt[:, esl], v_ps[:], AF.Tanh)
                        r0 = g * C + ct * 128
                        nc.sync.dma_start(out=v_spill[r0:r0 + 128, :], in_=vt[:])
                        for ns in range(E // 512):
                            esl = slice(ns * 512, (ns + 1) * 512)
                            nc.tensor.matmul(
                                kv_ps[:, esl], kl[:, ct, :], vt[:, esl],
                                start=(g == 0 and ct == 0),
                                stop=(g == G - 1 and ct == 1),
                            )
                    if g + 1 < G:
                        kls[g + 1] = z_back(g + 1, klTs.pop(g + 1))
                    del xgs[g]

                for ns in range(E // 512):
                    esl = slice(ns * 512, (ns + 1) * 512)
                    nc.vector.tensor_copy(kv_sb[:, esl], kv_ps[:, esl])

            # ================= PASS 2 =================
            with (
                tc.tile_pool(name="ps2", bufs=4, space="PSUM") as ps2,
                tc.tile_pool(name="po", bufs=2, space="PSUM") as po,
            ):
                for g in range(G):
                    xg = xp.tile([128, KD, C], BF16, tag="xg")
                    nc.sync.dma_start(out=xg[:], in_=xT_h[:, :, g * C:(g + 1) * C])
                    vts = []
                    for ct in range(2):
                        vt = vp.tile([128, E], BF16, tag="v")
                        r0 = g * C + ct * 128
                        nc.sync.dma_start(out=vt[:], in_=v_spill[r0:r0 + 128, :])
                        vts.append(vt)

                    # affines of z (all in [S, C] layout, per-partition consts)
                    qqT = small.tile([128, C], BF16, tag="qqT")
                    kqT = small.tile([128, C], BF16, tag="kqT")
                    qlT = small.tile([128, C], BF16, tag="qlT")
                    zg = zT_all[:, g, :]
                    nc.scalar.activation(qqT[:], zg, AF.Identity,
                                         bias=bet[:, 0:1], scale=gam[:, 0:1])
                    nc.scalar.activation(kqT[:], zg, AF.Identity,
                                         bias=bet[:, 1:2], scale=gam[:, 1:2])
                    nc.scalar.activation(qlT[:], zg, AF.Identity,
                                         bias=bet[:, 2:3], scale=gam[:, 2:3])

                    # a^T[m, n] = relu(qk^T[m, n] + rel_bias[m])^2
                    aT = small.tile([128, 2, C], BF16, tag="aT")
                    for mt in range(2):
                        qk_ps = ps2.tile([128, C], F32, tag="ps2")
                        nc.tensor.matmul(
                            qk_ps[:], kqT[:, mt * 128:(mt + 1) * 128], qqT[:],
                            start=True, stop=True,
                        )
                        rl = small.tile([128, C], BF16, tag="rl")
                        nc.scalar.activation(rl[:], qk_ps[:], AF.Relu,
                                             bias=relb[:, mt:mt + 1])
                        nc.vector.tensor_mul(aT[:, mt, :], rl[:], rl[:])

                    # per e-tile: (v_quad + v_lin)^T, u^T, h, o-accumulation
                    o_ps = [po.tile([128, D], F32, tag="o", name=f"o_ps{i}")
                            for i in range(2)]
                    for et in range(KE):
                        q_ps = ps2.tile([128, C], F32, tag="ps2")
                        etsl = slice(et * 128, (et + 1) * 128)
                        for mt in range(2):
                            nc.tensor.matmul(
                                q_ps[:], vts[mt][:, etsl], aT[:, mt, :],
                                start=(mt == 0), stop=False,
                            )
                        nc.tensor.matmul(
                            q_ps[:], kv_sb[:, etsl], qlT[:],
                            start=False, stop=True,
                        )
                        u_ps = ps2.tile([128, C], F32, tag="ps2")
                        for kt in range(KD):
                            nc.tensor.matmul(
                                u_ps[:], w_ue[:, kt, etsl], xg[:, kt, :],
                                start=(kt == 0), stop=(kt == KD - 1),
                            )
                        ut = small.tile([128, C], BF16, tag="ut")
                        nc.scalar.activation(ut[:], u_ps[:], AF.Tanh,
                                             bias=b_ue[:, et:et + 1])
                        ht = small.tile([128, C], BF16, tag="ht")
                        nc.vector.tensor_mul(ht[:], ut[:], q_ps[:])
                        for ct in range(2):
                            csl = slice(ct * 128, (ct + 1) * 128)
                            for ds in range(2):
                                dsl = slice(ds * 512, (ds + 1) * 512)
                                nc.tensor.matmul(
                                    o_ps[ct][:, dsl], ht[:, csl], w_od[:, et, dsl],
                                    start=(et == 0), stop=(et == KE - 1),
                                )

                    for ct in range(2):
                        ot = osb.tile([128, D], F32, tag="o_sb")
                        nc.vector.tensor_add(ot[:], o_ps[ct][:], b_od[:])
                        r0 = g * C + ct * 128
                        nc.sync.dma_start(out=o_h[r0:r0 + 128, :], in_=ot[:])

    return nc


_CACHE: dict = {}


def _prep_inputs(value, Wxs, bxs, Wve, bve, Wue, bue, Wod, bod,
                 rel_bias, gamma, beta):
    bf = ml_dtypes.bfloat16
    shared = {
        "Wxs": np.ascontiguousarray(
            Wxs.astype(bf).reshape(KD, 128, S).transpose(1, 0, 2)),
        "Wve": np.ascontiguousarray(
            Wve.astype(bf).reshape(KD, 128, E).transpose(1, 0, 2)),
        "Wue": np.ascontiguousarray(
            Wue.astype(bf).reshape(KD, 128, E).transpose(1, 0, 2)),
        "Wod": np.ascontiguousarray(
            Wod.astype(bf).reshape(KE, 128, D).transpose(1, 0, 2)),
        "bxs": np.ascontiguousarray(bxs.astype(np.float32).reshape(128, 1)),
        "bve": np.ascontiguousarray(
            np.broadcast_to(bve.astype(bf), (128, E))),
        "bue": np.ascontiguousarray(
            bue.astype(np.float32).reshape(KE, 128).T),
        "bod": np.ascontiguousarray(
            np.broadcast_to(bod.astype(np.float32), (128, D))),
        "rel": np.ascontiguousarray(
            rel_bias.astype(np.float32).reshape(2, 128).T),
        "gam": np.ascontiguousarray(gamma.astype(np.float32).T),
        "bet": np.ascontiguousarray(beta.astype(np.float32).T),
    }
    in_maps = []
    for b in range(B):
        xT = np.ascontiguousarray(value[:, b, :].T.astype(bf)
                                  .reshape(KD, 128, SRC).transpose(1, 0, 2))
        in_maps.append({"xT": xT, **shared})
    return in_maps


def kernel(**inputs) -> np.ndarray:
    inp = {k: np.asarray(v) for k, v in inputs.items()}
    in_maps = _prep_inputs(
        inp["value"], inp["Wxs"], inp["bxs"], inp["Wve"], inp["bve"],
        inp["Wue"], inp["bue"], inp["Wod"], inp["bod"],
        inp["rel_bias"], inp["gamma"], inp["beta"],
    )
    if "nc" not in _CACHE:
        nc = build_nc()
        nc.compile()
        _CACHE["nc"] = nc
    res = run_bass_kernel_spmd(_CACHE["nc"], in_maps, list(range(B))).results
    out = np.stack([r["o"] for r in res], axis=1)
    return np.ascontiguousarray(out.astype(np.float32))


# revision 33
# speedup vs baseline: 1.0160x; 1.0160x over previous
"""Trainium2 Bass kernel for nn_FLASHAttention_3650722201963.

Reference computation (per batch b, chunks g of size C=256 over SRC=4096):
    x = value[:, b, :]                      # [SRC, D]   (query/key are unused!)
    v = tanh(x @ Wve + bve)                 # [.., E]
    z = tanh(x @ Wxs + bxs)                 # [.., S]
    q_quad/k_quad/q_lin/k_lin = z * gamma_i + beta_i
    qk = q_quad @ k_quad^T (per chunk)      # [C, C]
    a  = relu(qk + rel_bias)^2
    v_quad = a @ v (per chunk)
    kv = sum_{g,c} k_lin^T v                # [S, E] global per batch
    v_lin = q_lin @ kv
    u = tanh(x @ Wue + bue)
    o = (u * (v_quad + v_lin)) @ Wod + bod  # [SRC, D]

Sharding: pure data parallel over batch (B=8) -> one batch element per core.
All matmuls in bf16 with fp32 PSUM accumulation. Host pre-transposes x to
xT [D, SRC] per core so the device never transposes activations; the weights'
natural [in, out] layout is already what the PE wants.

Two passes per core:
  pass 1: zT (kept resident), k_lin (via PE transpose), v (spilled bf16 to
          DRAM scratch), kv accumulated across all chunks in a resident PSUM
          tile.
  pass 2: qk^T -> a^T (rel_bias folds into the ACT as a per-partition bias),
          per e-tile: v_quad^T and v_lin^T accumulate into the SAME psum bank,
          u^T = tanh(.. + bue) (per-partition bias), h = u^T*(vq+vl) on DVE,
          o = h^T-matmuls accumulated over e in PSUM, bod added during the
          PSUM->SBUF copy.
"""

import numpy as np
import ml_dtypes

import concourse.bass as bass
import concourse.tile as tile
from concourse.tile import add_dep_helper
from concourse import bacc, mybir
from concourse.bass_utils import run_bass_kernel_spmd
from concourse.masks import make_identity

BF16 = mybir.dt.bfloat16
F32 = mybir.dt.float32
AF = mybir.ActivationFunctionType

D = 1024      # embed dim
E = 2048      # expanded dim
S = 128       # shrunken attn dim
C = 256       # chunk size
SRC = 4096    # sequence length
G = SRC // C  # 16 chunks
B = 8         # batch == n cores
KD = D // 128   # 8 k-tiles over D
KE = E // 128   # 16 e-tiles


def build_nc() -> bacc.Bacc:
    nc = bacc.Bacc(None, target_bir_lowering=False, debug=False)

    # ---- I/O ----
    xT_h = nc.declare_dram_parameter("xT", [128, KD, SRC], BF16, isOutput=False)
    wxs_h = nc.declare_dram_parameter("Wxs", [128, KD, S], BF16, isOutput=False)
    wve_h = nc.declare_dram_parameter("Wve", [128, KD, E], BF16, isOutput=False)
    wue_h = nc.declare_dram_parameter("Wue", [128, KD, E], BF16, isOutput=False)
    wod_h = nc.declare_dram_parameter("Wod", [128, KE, D], BF16, isOutput=False)
    bxs_h = nc.declare_dram_parameter("bxs", [128, 1], F32, isOutput=False)
    bve_h = nc.declare_dram_parameter("bve", [128, E], BF16, isOutput=False)
    bue_h = nc.declare_dram_parameter("bue", [128, KE], F32, isOutput=False)
    bod_h = nc.declare_dram_parameter("bod", [128, D], F32, isOutput=False)
    rel_h = nc.declare_dram_parameter("rel", [128, 2], F32, isOutput=False)
    gam_h = nc.declare_dram_parameter("gam", [128, 4], F32, isOutput=False)
    bet_h = nc.declare_dram_parameter("bet", [128, 4], F32, isOutput=False)
    o_h = nc.declare_dram_parameter("o", [SRC, D], F32, isOutput=True)

    v_spill = nc.dram_tensor("v_spill", [SRC, E], BF16)

    with tile.TileContext(nc) as tc:
        with (
            tc.tile_pool(name="consts", bufs=1) as consts,
            tc.tile_pool(name="xp", bufs=3) as xp,
            tc.tile_pool(name="vp", bufs=4) as vp,
            tc.tile_pool(name="small", bufs=3) as small,
            tc.tile_pool(name="op", bufs=3) as osb,
        ):
            # ---- resident constants ----
            w_xs = consts.tile([128, KD, S], BF16)
            w_ve = consts.tile([128, KD, E], BF16)
            w_ue = consts.tile([128, KD, E], BF16)
            w_od = consts.tile([128, KE, D], BF16)
            b_xs = consts.tile([128, 1], F32)
            b_ve = consts.tile([128, E], BF16)
            b_ue = consts.tile([128, KE], F32)
            b_od = consts.tile([128, D], F32)
            relb = consts.tile([128, 2], F32)
            gam = consts.tile([128, 4], F32)
            bet = consts.tile([128, 4], F32)
            ident = consts.tile([128, 128], BF16)
            zT_all = consts.tile([128, G, C], BF16)
            kv_sb = consts.tile([128, E], BF16)

            # HAM warmup + DMA-window cover: ~300 dummy matmuls (~18us) keep
            # the PE busy while the ~6MB of first-touch weights stream from
            # HBM. Without this the PE idles in 3-7us holes waiting for Wve
            # slices, re-throttling the clock to 1.2GHz (HAM) and running
            # the first ~30us of real matmuls at half rate.
            with tc.tile_pool(name="pwarm", bufs=1, space="PSUM") as pwarm:
                warm_in = consts.tile([128, 128], BF16)
                nc.vector.memset(warm_in[:], 0.0)
                warm_ps = pwarm.tile([128, 128], F32)
                for _ in range(140):
                    nc.tensor.matmul(warm_ps[:], warm_in[:], warm_in[:],
                                     start=True, stop=True)

            # Head-latency critical path: the sync HWDGE ring carries only
            # what the first chunk needs, in need-order (Wxs -> x chunk 0 ->
            # Wve streamed per k-tile). Small pass-1 consts ride the scalar
            # ring early; the 8.5MB of pass-2-only weights are emitted later
            # (inside the pass-1 loop) so DMA-semaphore recycling never makes
            # a critical load wait on a big transfer.
            nc.sync.dma_start(out=w_xs[:], in_=wxs_h[:])
            nc.scalar.dma_start(out=b_xs[:], in_=bxs_h[:])
            nc.scalar.dma_start(out=gam[:], in_=gam_h[:])
            nc.scalar.dma_start(out=bet[:], in_=bet_h[:])
            nc.scalar.dma_start(out=b_ve[:], in_=bve_h[:])
            make_identity(nc, ident[:])

            # ================= PASS 1 =================
            with (
                tc.tile_pool(name="psz", bufs=2, space="PSUM") as psz,
                tc.tile_pool(name="psv", bufs=2, space="PSUM") as psv,
                tc.tile_pool(name="pkv", bufs=1, space="PSUM") as pkv,
            ):
                kv_ps = pkv.tile([128, E], F32)

                xgs: dict = {}

                def get_xg(g):
                    if g not in xgs:
                        t = xp.tile([128, KD, C], BF16, tag="xg",
                                    name=f"xg{g}")
                        sl = slice(g * C, (g + 1) * C)
                        nc.sync.dma_start(out=t[:], in_=xT_h[:, :, sl])
                        xgs[g] = t
                    return xgs[g]

                def z_front(g):
                    """zT[g] = tanh(Wxs^T x^T + bxs) + k_lin^T affine.
                    Emitted one chunk ahead of its v-block so the PE->ACT
                    round trip hides under the previous chunk's matmuls."""
                    xg = get_xg(g)
                    z_ps = psz.tile([128, C], F32, tag="zps", name=f"zps{g}")
                    for kt in range(KD):
                        nc.tensor.matmul(
                            z_ps[:], w_xs[:, kt, :], xg[:, kt, :],
                            start=(kt == 0), stop=(kt == KD - 1),
                        )
                    nc.scalar.activation(
                        zT_all[:, g, :], z_ps[:], AF.Tanh, bias=b_xs[:, 0:1],
                    )
                    klT = small.tile([128, C], BF16, tag="klT",
                                     name=f"klT{g}")
                    nc.scalar.activation(
                        klT[:], zT_all[:, g, :], AF.Identity,
                        bias=bet[:, 3:4], scale=gam[:, 3:4],
                    )
                    return klT

                def z_back(g, klT):
                    """PE-transpose k_lin^T -> k_lin [C, S]; emitted after the
                    previous chunk's matmul block so the PE never waits on the
                    ACT affine."""
                    kl = small.tile([128, 2, S], BF16, tag="kl",
                                    name=f"kl{g}")
                    for ct in range(2):
                        t_ps = psz.tile([128, 128], BF16, tag="zps",
                                        name=f"tps{g}_{ct}")
                        nc.tensor.transpose(
                            t_ps[:], klT[:, ct * 128:(ct + 1) * 128], ident[:]
                        )
                        nc.vector.tensor_copy(kl[:, ct, :], t_ps[:])
                    return kl

                get_xg(0)
                # ns-major streaming matches the v-loop's consumption order:
                # the (ct0, ns0) psum group needs all k-tiles of e-cols 0:512.
                # xg1 is prefetched after the first slice so chunk 1's zT
                # isn't stuck behind the remaining 3MB of Wve.
                for ns in range(E // 512):
                    esl = slice(ns * 512, (ns + 1) * 512)
                    nc.sync.dma_start(out=w_ve[:, :, esl], in_=wve_h[:, :, esl])
                    if ns == 0:
                        get_xg(1)
                kls = {0: z_back(0, z_front(0))}
                klTs: dict = {}
                for g in range(G):
                    xg = get_xg(g)
                    if g + 1 < G:
                        klTs[g + 1] = z_front(g + 1)
                    kl = kls.pop(g)

                    # v = tanh(x @ Wve + bve), spill to DRAM; kv += k_lin^T v
                    for ct in range(2):
                        vt = vp.tile([128, E], BF16, tag="v")
                        for ns in range(E // 512):
                            v_ps = psv.tile([128, 512], F32, tag="vps")
                            esl = slice(ns * 512, (ns + 1) * 512)
                            for kt in range(KD):
                                nc.tensor.matmul(
                                    v_ps[:], xg[:, kt, ct * 128:(ct + 1) * 128],
                                    w_ve[:, kt, esl],
                                    start=(kt == 0), stop=(kt == KD - 1),
                                )
                            nc.vector.tensor_add(v_ps[:], v_ps[:], b_ve[:, esl])
                            nc.scalar.activation(vt[:, esl], v_ps[:], AF.Tanh)
                        r0 = g * C + ct * 128
                        nc.sync.dma_start(out=v_spill[r0:r0 + 128, :], in_=vt[:])
                        for ns in range(E // 512):
                            esl = slice(ns * 512, (ns + 1) * 512)
                            mm = nc.tensor.matmul(
                                kv_ps[:, esl], kl[:, ct, :], vt[:, esl],
                                start=(g == 0 and ct == 0),
                                stop=(g == G - 1 and ct == 1),
                            )
                            if g == 0 and ct == 0 and ns == 0:
                                kv_gate = mm
                    if g == 0:
                        # Pass-2-only weights, explicitly gated behind the
                        # first kv matmul: without the dep the scheduler
                        # hoists these no-dependency DMAs to t~10us, where
                        # their 9MB halves Wve's streaming bandwidth and
                        # stalls the first v-matmul wave until ~37us.
                        for dst, src in ((w_ue, wue_h), (w_od, wod_h),
                                         (b_ue, bue_h), (b_od, bod_h),
                                         (relb, rel_h)):
                            di = nc.sync.dma_start(out=dst[:], in_=src[:])
                            add_dep_helper(di.ins, kv_gate.ins, sync=True,
                                           reason="defer pass-2 weights")
                    if g + 1 < G:
                        kls[g + 1] = z_back(g + 1, klTs.pop(g + 1))
                    del xgs[g]

                for ns in range(E // 512):
                    esl = slice(ns * 512, (ns + 1) * 512)
                    nc.vector.tensor_copy(kv_sb[:, esl], kv_ps[:, esl])

            # ================= PASS 2 =================
            with (
                tc.tile_pool(name="ps2", bufs=4, space="PSUM") as ps2,
                tc.tile_pool(name="po", bufs=2, space="PSUM") as po,
            ):
                for g in range(G):
                    xg = xp.tile([128, KD, C], BF16, tag="xg")
                    nc.sync.dma_start(out=xg[:], in_=xT_h[:, :, g * C:(g + 1) * C])
                    vts = []
                    for ct in range(2):
                        vt = vp.tile([128, E], BF16, tag="v")
                        r0 = g * C + ct * 128
                        nc.sync.dma_start(out=vt[:], in_=v_spill[r0:r0 + 128, :])
                        vts.append(vt)

                    # affines of z (all in [S, C] layout, per-partition consts)
                    qqT = small.tile([128, C], BF16, tag="qqT")
                    kqT = small.tile([128, C], BF16, tag="kqT")
                    qlT = small.tile([128, C], BF16, tag="qlT")
                    zg = zT_all[:, g, :]
                    nc.scalar.activation(qqT[:], zg, AF.Identity,
                                         bias=bet[:, 0:1], scale=gam[:, 0:1])
                    nc.scalar.activation(kqT[:], zg, AF.Identity,
                                         bias=bet[:, 1:2], scale=gam[:, 1:2])
                    nc.scalar.activation(qlT[:], zg, AF.Identity,
                                         bias=bet[:, 2:3], scale=gam[:, 2:3])

                    # a^T[m, n] = relu(qk^T[m, n] + rel_bias[m])^2
                    aT = small.tile([128, 2, C], BF16, tag="aT")
                    for mt in range(2):
                        qk_ps = ps2.tile([128, C], F32, tag="ps2")
                        nc.tensor.matmul(
                            qk_ps[:], kqT[:, mt * 128:(mt + 1) * 128], qqT[:],
                            start=True, stop=True,
                        )
                        rl = small.tile([128, C], BF16, tag="rl")
                        nc.scalar.activation(rl[:], qk_ps[:], AF.Relu,
                                             bias=relb[:, mt:mt + 1])
                        nc.vector.tensor_mul(aT[:, mt, :], rl[:], rl[:])

                    # per e-tile: (v_quad + v_lin)^T, u^T, h, o-accumulation
                    o_ps = [po.tile([128, D], F32, tag="o", name=f"o_ps{i}")
                            for i in range(2)]
                    for et in range(KE):
                        q_ps = ps2.tile([128, C], F32, tag="ps2")
                        etsl = slice(et * 128, (et + 1) * 128)
                        for mt in range(2):
                            nc.tensor.matmul(
                                q_ps[:], vts[mt][:, etsl], aT[:, mt, :],
                                start=(mt == 0), stop=False,
                            )
                        nc.tensor.matmul(
                            q_ps[:], kv_sb[:, etsl], qlT[:],
                            start=False, stop=True,
                        )
                        u_ps = ps2.tile([128, C], F32, tag="ps2")
                        for kt in range(KD):
                            nc.tensor.matmul(
                                u_ps[:], w_ue[:, kt, etsl], xg[:, kt, :],
                                start=(kt == 0), stop=(kt == KD - 1),
                            )
                        ut = small.tile([128, C], BF16, tag="ut")
                        nc.scalar.activation(ut[:], u_ps[:], AF.Tanh,
                                             bias=b_ue[:, et:et + 1])
                        ht = small.tile([128, C], BF16, tag="ht")
                        nc.vector.tensor_mul(ht[:], ut[:], q_ps[:])
                        for ct in range(2):
                            csl = slice(ct * 128, (ct + 1) * 128)
                            for ds in range(2):
                                dsl = slice(ds * 512, (ds + 1) * 512)
                                nc.tensor.matmul(
                                    o_ps[ct][:, dsl], ht[:, csl], w_od[:, et, dsl],
                                    start=(et == 0), stop=(et == KE - 1),
                                )

                    for ct in range(2):
                        ot = osb.tile([128, D], F32, tag="o_sb")
                        nc.vector.tensor_add(ot[:], o_ps[ct][:], b_od[:])
                        r0 = g * C + ct * 128
                        nc.sync.dma_start(out=o_h[r0:r0 + 128, :], in_=ot[:])

    return nc


_CACHE: dict = {}


def _prep_inputs(value, Wxs, bxs, Wve, bve, Wue, bue, Wod, bod,
                 rel_bias, gamma, beta):
    bf = ml_dtypes.bfloat16
    shared = {
        "Wxs": np.ascontiguousarray(
            Wxs.astype(bf).reshape(KD, 128, S).transpose(1, 0, 2)),
        "Wve": np.ascontiguousarray(
            Wve.astype(bf).reshape(KD, 128, E).transpose(1, 0, 2)),
        "Wue": np.ascontiguousarray(
            Wue.astype(bf).reshape(KD, 128, E).transpose(1, 0, 2)),
        "Wod": np.ascontiguousarray(
            Wod.astype(bf).reshape(KE, 128, D).transpose(1, 0, 2)),
        "bxs": np.ascontiguousarray(bxs.astype(np.float32).reshape(128, 1)),
        "bve": np.ascontiguousarray(
            np.broadcast_to(bve.astype(bf), (128, E))),
        "bue": np.ascontiguousarray(
            bue.astype(np.float32).reshape(KE, 128).T),
        "bod": np.ascontiguousarray(
            np.broadcast_to(bod.astype(np.float32), (128, D))),
        "rel": np.ascontiguousarray(
            rel_bias.astype(np.float32).reshape(2, 128).T),
        "gam": np.ascontiguousarray(gamma.astype(np.float32).T),
        "bet": np.ascontiguousarray(beta.astype(np.float32).T),
    }
    in_maps = []
    for b in range(B):
        xT = np.ascontiguousarray(value[:, b, :].T.astype(bf)
                                  .reshape(KD, 128, SRC).transpose(1, 0, 2))
        in_maps.append({"xT": xT, **shared})
    return in_maps


def kernel(**inputs) -> np.ndarray:
    inp = {k: np.asarray(v) for k, v in inputs.items()}
    in_maps = _prep_inputs(
        inp["value"], inp["Wxs"], inp["bxs"], inp["Wve"], inp["bve"],
        inp["Wue"], inp["bue"], inp["Wod"], inp["bod"],
        inp["rel_bias"], inp["gamma"], inp["beta"],
    )
    if "nc" not in _CACHE:
        nc = build_nc()
        nc.compile()
        _CACHE["nc"] = nc
    res = run_bass_kernel_spmd(_CACHE["nc"], in_maps, list(range(B))).results
    out = np.stack([r["o"] for r in res], axis=1)
    return np.ascontiguousarray(out.astype(np.float32))


# revision 34
# speedup vs baseline: 1.0255x; 1.0094x over previous
"""Trainium2 Bass kernel for nn_FLASHAttention_3650722201963.

Reference computation (per batch b, chunks g of size C=256 over SRC=4096):
    x = value[:, b, :]                      # [SRC, D]   (query/key are unused!)
    v = tanh(x @ Wve + bve)                 # [.., E]
    z = tanh(x @ Wxs + bxs)                 # [.., S]
    q_quad/k_quad/q_lin/k_lin = z * gamma_i + beta_i
    qk = q_quad @ k_quad^T (per chunk)      # [C, C]
    a  = relu(qk + rel_bias)^2
    v_quad = a @ v (per chunk)
    kv = sum_{g,c} k_lin^T v                # [S, E] global per batch
    v_lin = q_lin @ kv
    u = tanh(x @ Wue + bue)
    o = (u * (v_quad + v_lin)) @ Wod + bod  # [SRC, D]

Sharding: pure data parallel over batch (B=8) -> one batch element per core.
All matmuls in bf16 with fp32 PSUM accumulation. Host pre-transposes x to
xT [D, SRC] per core so the device never transposes activations; the weights'
natural [in, out] layout is already what the PE wants.

Two passes per core:
  pass 1: zT (kept resident), k_lin (via PE transpose), v (spilled bf16 to
          DRAM scratch), kv accumulated across all chunks in a resident PSUM
          tile.
  pass 2: qk^T -> a^T (rel_bias folds into the ACT as a per-partition bias),
          per e-tile: v_quad^T and v_lin^T accumulate into the SAME psum bank,
          u^T = tanh(.. + bue) (per-partition bias), h = u^T*(vq+vl) on DVE,
          o = h^T-matmuls accumulated over e in PSUM, bod added during the
          PSUM->SBUF copy.
"""

import numpy as np
import ml_dtypes

import concourse.bass as bass
import concourse.tile as tile
from concourse.tile import add_dep_helper
from concourse import bacc, mybir
from concourse.bass_utils import run_bass_kernel_spmd
from concourse.masks import make_identity

BF16 = mybir.dt.bfloat16
F32 = mybir.dt.float32
AF = mybir.ActivationFunctionType

D = 1024      # embed dim
E = 2048      # expanded dim
S = 128       # shrunken attn dim
C = 256       # chunk size
SRC = 4096    # sequence length
G = SRC // C  # 16 chunks
B = 8         # batch == n cores
KD = D // 128   # 8 k-tiles over D
KE = E // 128   # 16 e-tiles


def build_nc() -> bacc.Bacc:
    nc = bacc.Bacc(None, target_bir_lowering=False, debug=False)

    # ---- I/O ----
    xT_h = nc.declare_dram_parameter("xT", [128, KD, SRC], BF16, isOutput=False)
    wxs_h = nc.declare_dram_parameter("Wxs", [128, KD, S], BF16, isOutput=False)
    wve_h = nc.declare_dram_parameter("Wve", [128, KD, E], BF16, isOutput=False)
    wue_h = nc.declare_dram_parameter("Wue", [128, KD, E], BF16, isOutput=False)
    wod_h = nc.declare_dram_parameter("Wod", [128, KE, D], BF16, isOutput=False)
    bxs_h = nc.declare_dram_parameter("bxs", [128, 1], F32, isOutput=False)
    bve_h = nc.declare_dram_parameter("bve", [128, E], BF16, isOutput=False)
    bue_h = nc.declare_dram_parameter("bue", [128, KE], F32, isOutput=False)
    bod_h = nc.declare_dram_parameter("bod", [128, D], F32, isOutput=False)
    rel_h = nc.declare_dram_parameter("rel", [128, 2], F32, isOutput=False)
    gam_h = nc.declare_dram_parameter("gam", [128, 4], F32, isOutput=False)
    bet_h = nc.declare_dram_parameter("bet", [128, 4], F32, isOutput=False)
    o_h = nc.declare_dram_parameter("o", [SRC, D], F32, isOutput=True)

    v_spill = nc.dram_tensor("v_spill", [SRC, E], BF16)

    with tile.TileContext(nc) as tc:
        with (
            tc.tile_pool(name="consts", bufs=1) as consts,
            tc.tile_pool(name="xp", bufs=3) as xp,
            tc.tile_pool(name="vp", bufs=4) as vp,
            tc.tile_pool(name="small", bufs=3) as small,
            tc.tile_pool(name="op", bufs=3) as osb,
        ):
            # ---- resident constants ----
            w_xs = consts.tile([128, KD, S], BF16)
            w_ve = consts.tile([128, KD, E], BF16)
            w_ue = consts.tile([128, KD, E], BF16)
            w_od = consts.tile([128, KE, D], BF16)
            b_xs = consts.tile([128, 1], F32)
            b_ve = consts.tile([128, E], BF16)
            b_ue = consts.tile([128, KE], F32)
            b_od = consts.tile([128, D], F32)
            relb = consts.tile([128, 2], F32)
            gam = consts.tile([128, 4], F32)
            bet = consts.tile([128, 4], F32)
            ident = consts.tile([128, 128], BF16)
            zT_all = consts.tile([128, G, C], BF16)
            kv_sb = consts.tile([128, E], BF16)

            # HAM warmup + DMA-window cover: ~300 dummy matmuls (~18us) keep
            # the PE busy while the ~6MB of first-touch weights stream from
            # HBM. Without this the PE idles in 3-7us holes waiting for Wve
            # slices, re-throttling the clock to 1.2GHz (HAM) and running
            # the first ~30us of real matmuls at half rate.
            with tc.tile_pool(name="pwarm", bufs=1, space="PSUM") as pwarm:
                warm_in = consts.tile([128, 128], BF16)
                nc.vector.memset(warm_in[:], 0.0)
                warm_ps = pwarm.tile([128, 128], F32)
                for _ in range(140):
                    nc.tensor.matmul(warm_ps[:], warm_in[:], warm_in[:],
                                     start=True, stop=True)

            # Head-latency critical path: the sync HWDGE ring carries only
            # what the first chunk needs, in need-order (Wxs -> x chunk 0 ->
            # Wve streamed per k-tile). Small pass-1 consts ride the scalar
            # ring early; the 8.5MB of pass-2-only weights are emitted later
            # (inside the pass-1 loop) so DMA-semaphore recycling never makes
            # a critical load wait on a big transfer.
            nc.sync.dma_start(out=w_xs[:], in_=wxs_h[:])
            nc.scalar.dma_start(out=b_xs[:], in_=bxs_h[:])
            nc.scalar.dma_start(out=gam[:], in_=gam_h[:])
            nc.scalar.dma_start(out=bet[:], in_=bet_h[:])
            nc.scalar.dma_start(out=b_ve[:], in_=bve_h[:])
            make_identity(nc, ident[:])

            # ================= PASS 1 =================
            with (
                tc.tile_pool(name="psz", bufs=2, space="PSUM") as psz,
                tc.tile_pool(name="psv", bufs=2, space="PSUM") as psv,
                tc.tile_pool(name="pkv", bufs=1, space="PSUM") as pkv,
            ):
                kv_ps = pkv.tile([128, E], F32)

                xgs: dict = {}

                def get_xg(g):
                    if g not in xgs:
                        t = xp.tile([128, KD, C], BF16, tag="xg",
                                    name=f"xg{g}")
                        sl = slice(g * C, (g + 1) * C)
                        nc.sync.dma_start(out=t[:], in_=xT_h[:, :, sl])
                        xgs[g] = t
                    return xgs[g]

                def z_front(g):
                    """zT[g] = tanh(Wxs^T x^T + bxs) + k_lin^T affine.
                    Emitted one chunk ahead of its v-block so the PE->ACT
                    round trip hides under the previous chunk's matmuls."""
                    xg = get_xg(g)
                    z_ps = psz.tile([128, C], F32, tag="zps", name=f"zps{g}")
                    for kt in range(KD):
                        nc.tensor.matmul(
                            z_ps[:], w_xs[:, kt, :], xg[:, kt, :],
                            start=(kt == 0), stop=(kt == KD - 1),
                        )
                    nc.scalar.activation(
                        zT_all[:, g, :], z_ps[:], AF.Tanh, bias=b_xs[:, 0:1],
                    )
                    klT = small.tile([128, C], BF16, tag="klT",
                                     name=f"klT{g}")
                    nc.scalar.activation(
                        klT[:], zT_all[:, g, :], AF.Identity,
                        bias=bet[:, 3:4], scale=gam[:, 3:4],
                    )
                    return klT

                def z_back(g, klT):
                    """PE-transpose k_lin^T -> k_lin [C, S]; emitted after the
                    previous chunk's matmul block so the PE never waits on the
                    ACT affine."""
                    kl = small.tile([128, 2, S], BF16, tag="kl",
                                    name=f"kl{g}")
                    for ct in range(2):
                        t_ps = psz.tile([128, 128], BF16, tag="zps",
                                        name=f"tps{g}_{ct}")
                        nc.tensor.transpose(
                            t_ps[:], klT[:, ct * 128:(ct + 1) * 128], ident[:]
                        )
                        nc.vector.tensor_copy(kl[:, ct, :], t_ps[:])
                    return kl

                get_xg(0)
                # ns-major streaming matches the v-loop's consumption order:
                # the (ct0, ns0) psum group needs all k-tiles of e-cols 0:512.
                # xg1 is prefetched after the first slice so chunk 1's zT
                # isn't stuck behind the remaining 3MB of Wve.
                for ns in range(E // 512):
                    esl = slice(ns * 512, (ns + 1) * 512)
                    nc.sync.dma_start(out=w_ve[:, :, esl], in_=wve_h[:, :, esl])
                    if ns == 0:
                        get_xg(1)
                kls = {0: z_back(0, z_front(0))}
                klTs: dict = {}
                for g in range(G):
                    xg = get_xg(g)
                    if g + 1 < G:
                        klTs[g + 1] = z_front(g + 1)
                    kl = kls.pop(g)

                    # v = tanh(x @ Wve + bve), spill to DRAM; kv += k_lin^T v
                    for ct in range(2):
                        vt = vp.tile([128, E], BF16, tag="v")
                        for ns in range(E // 512):
                            v_ps = psv.tile([128, 512], F32, tag="vps")
                            esl = slice(ns * 512, (ns + 1) * 512)
                            for kt in range(KD):
                                nc.tensor.matmul(
                                    v_ps[:], xg[:, kt, ct * 128:(ct + 1) * 128],
                                    w_ve[:, kt, esl],
                                    start=(kt == 0), stop=(kt == KD - 1),
                                )
                            nc.vector.tensor_add(v_ps[:], v_ps[:], b_ve[:, esl])
                            nc.scalar.activation(vt[:, esl], v_ps[:], AF.Tanh)
                        r0 = g * C + ct * 128
                        nc.sync.dma_start(out=v_spill[r0:r0 + 128, :], in_=vt[:])
                        for ns in range(E // 512):
                            esl = slice(ns * 512, (ns + 1) * 512)
                            mm = nc.tensor.matmul(
                                kv_ps[:, esl], kl[:, ct, :], vt[:, esl],
                                start=(g == 0 and ct == 0),
                                stop=(g == G - 1 and ct == 1),
                            )
                            if ct == 0 and ns == 0:
                                kv_gate = mm
                    # Pass-2-only weights trickled in ~1MB slices, each
                    # gated behind this chunk's kv matmul. Without the dep
                    # the scheduler hoists the no-dependency DMAs to t~10us
                    # where 9MB halves Wve's bandwidth (first v-wave stalls
                    # to ~37us); one big gated burst instead starves the
                    # xg3 load behind 8.5MB of FIFO (5.6us hole at ~56us).
                    es2 = E // 4
                    if g < 4:        # w_ue in 4 column slices
                        di = nc.sync.dma_start(
                            out=w_ue[:, :, g * es2:(g + 1) * es2],
                            in_=wue_h[:, :, g * es2:(g + 1) * es2])
                    elif g < 8:      # w_od in 4 e-tile-row slices
                        j = g - 4
                        di = nc.sync.dma_start(
                            out=w_od[:, j * 4:(j + 1) * 4, :],
                            in_=wod_h[:, j * 4:(j + 1) * 4, :])
                    elif g == 8:
                        for dst, src in ((b_ue, bue_h), (b_od, bod_h),
                                         (relb, rel_h)):
                            di = nc.sync.dma_start(out=dst[:], in_=src[:])
                            add_dep_helper(di.ins, kv_gate.ins, sync=True,
                                           reason="defer pass-2 consts")
                        di = None
                    else:
                        di = None
                    if di is not None:
                        add_dep_helper(di.ins, kv_gate.ins, sync=True,
                                       reason="defer pass-2 weights")
                    if g + 1 < G:
                        kls[g + 1] = z_back(g + 1, klTs.pop(g + 1))
                    del xgs[g]

                for ns in range(E // 512):
                    esl = slice(ns * 512, (ns + 1) * 512)
                    nc.vector.tensor_copy(kv_sb[:, esl], kv_ps[:, esl])

            # ================= PASS 2 =================
            with (
                tc.tile_pool(name="ps2", bufs=4, space="PSUM") as ps2,
                tc.tile_pool(name="po", bufs=2, space="PSUM") as po,
            ):
                for g in range(G):
                    xg = xp.tile([128, KD, C], BF16, tag="xg")
                    nc.sync.dma_start(out=xg[:], in_=xT_h[:, :, g * C:(g + 1) * C])
                    vts = []
                    for ct in range(2):
                        vt = vp.tile([128, E], BF16, tag="v")
                        r0 = g * C + ct * 128
                        nc.sync.dma_start(out=vt[:], in_=v_spill[r0:r0 + 128, :])
                        vts.append(vt)

                    # affines of z (all in [S, C] layout, per-partition consts)
                    qqT = small.tile([128, C], BF16, tag="qqT")
                    kqT = small.tile([128, C], BF16, tag="kqT")
                    qlT = small.tile([128, C], BF16, tag="qlT")
                    zg = zT_all[:, g, :]
                    nc.scalar.activation(qqT[:], zg, AF.Identity,
                                         bias=bet[:, 0:1], scale=gam[:, 0:1])
                    nc.scalar.activation(kqT[:], zg, AF.Identity,
                                         bias=bet[:, 1:2], scale=gam[:, 1:2])
                    nc.scalar.activation(qlT[:], zg, AF.Identity,
                                         bias=bet[:, 2:3], scale=gam[:, 2:3])

                    # a^T[m, n] = relu(qk^T[m, n] + rel_bias[m])^2
                    aT = small.tile([128, 2, C], BF16, tag="aT")
                    for mt in range(2):
                        qk_ps = ps2.tile([128, C], F32, tag="ps2")
                        nc.tensor.matmul(
                            qk_ps[:], kqT[:, mt * 128:(mt + 1) * 128], qqT[:],
                            start=True, stop=True,
                        )
                        rl = small.tile([128, C], BF16, tag="rl")
                        nc.scalar.activation(rl[:], qk_ps[:], AF.Relu,
                                             bias=relb[:, mt:mt + 1])
                        nc.vector.tensor_mul(aT[:, mt, :], rl[:], rl[:])

                    # per e-tile: (v_quad + v_lin)^T, u^T, h, o-accumulation
                    o_ps = [po.tile([128, D], F32, tag="o", name=f"o_ps{i}")
                            for i in range(2)]
                    for et in range(KE):
                        q_ps = ps2.tile([128, C], F32, tag="ps2")
                        etsl = slice(et * 128, (et + 1) * 128)
                        for mt in range(2):
                            nc.tensor.matmul(
                                q_ps[:], vts[mt][:, etsl], aT[:, mt, :],
                                start=(mt == 0), stop=False,
                            )
                        nc.tensor.matmul(
                            q_ps[:], kv_sb[:, etsl], qlT[:],
                            start=False, stop=True,
                        )
                        u_ps = ps2.tile([128, C], F32, tag="ps2")
                        for kt in range(KD):
                            nc.tensor.matmul(
                                u_ps[:], w_ue[:, kt, etsl], xg[:, kt, :],
                                start=(kt == 0), stop=(kt == KD - 1),
                            )
                        ut = small.tile([128, C], BF16, tag="ut")
                        nc.scalar.activation(ut[:], u_ps[:], AF.Tanh,
                                             bias=b_ue[:, et:et + 1])
                        ht = small.tile([128, C], BF16, tag="ht")
                        nc.vector.tensor_mul(ht[:], ut[:], q_ps[:])
                        for ct in range(2):
                            csl = slice(ct * 128, (ct + 1) * 128)
                            for ds in range(2):
                                dsl = slice(ds * 512, (ds + 1) * 512)
                                nc.tensor.matmul(
                                    o_ps[ct][:, dsl], ht[:, csl], w_od[:, et, dsl],
                                    start=(et == 0), stop=(et == KE - 1),
                                )

                    for ct in range(2):
                        ot = osb.tile([128, D], F32, tag="o_sb")
                        nc.vector.tensor_add(ot[:], o_ps[ct][:], b_od[:])
                        r0 = g * C + ct * 128
                        nc.sync.dma_start(out=o_h[r0:r0 + 128, :], in_=ot[:])

    return nc


_CACHE: dict = {}


def _prep_inputs(value, Wxs, bxs, Wve, bve, Wue, bue, Wod, bod,
                 rel_bias, gamma, beta):
    bf = ml_dtypes.bfloat16
    shared = {
        "Wxs": np.ascontiguousarray(
            Wxs.astype(bf).reshape(KD, 128, S).transpose(1, 0, 2)),
        "Wve": np.ascontiguousarray(
            Wve.astype(bf).reshape(KD, 128, E).transpose(1, 0, 2)),
        "Wue": np.ascontiguousarray(
            Wue.astype(bf).reshape(KD, 128, E).transpose(1, 0, 2)),
        "Wod": np.ascontiguousarray(
            Wod.astype(bf).reshape(KE, 128, D).transpose(1, 0, 2)),
        "bxs": np.ascontiguousarray(bxs.astype(np.float32).reshape(128, 1)),
        "bve": np.ascontiguousarray(
            np.broadcast_to(bve.astype(bf), (128, E))),
        "bue": np.ascontiguousarray(
            bue.astype(np.float32).reshape(KE, 128).T),
        "bod": np.ascontiguousarray(
            np.broadcast_to(bod.astype(np.float32), (128, D))),
        "rel": np.ascontiguousarray(
            rel_bias.astype(np.float32).reshape(2, 128).T),
        "gam": np.ascontiguousarray(gamma.astype(np.float32).T),
        "bet": np.ascontiguousarray(beta.astype(np.float32).T),
    }
    in_maps = []
    for b in range(B):
        xT = np.ascontiguousarray(value[:, b, :].T.astype(bf)
                                  .reshape(KD, 128, SRC).transpose(1, 0, 2))
        in_maps.append({"xT": xT, **shared})
    return in_maps


def kernel(**inputs) -> np.ndarray:
    inp = {k: np.asarray(v) for k, v in inputs.items()}
    in_maps = _prep_inputs(
        inp["value"], inp["Wxs"], inp["bxs"], inp["Wve"], inp["bve"],
        inp["Wue"], inp["bue"], inp["Wod"], inp["bod"],
        inp["rel_bias"], inp["gamma"], inp["beta"],
    )
    if "nc" not in _CACHE:
        nc = build_nc()
        nc.compile()
        _CACHE["nc"] = nc
    res = run_bass_kernel_spmd(_CACHE["nc"], in_maps, list(range(B))).results
    out = np.stack([r["o"] for r in res], axis=1)
    return np.ascontiguousarray(out.astype(np.float32))


# revision 38
# speedup vs baseline: 1.0271x; 1.0015x over previous
"""Trainium2 Bass kernel for nn_FLASHAttention_3650722201963.

Reference computation (per batch b, chunks g of size C=256 over SRC=4096):
    x = value[:, b, :]                      # [SRC, D]   (query/key are unused!)
    v = tanh(x @ Wve + bve)                 # [.., E]
    z = tanh(x @ Wxs + bxs)                 # [.., S]
    q_quad/k_quad/q_lin/k_lin = z * gamma_i + beta_i
    qk = q_quad @ k_quad^T (per chunk)      # [C, C]
    a  = relu(qk + rel_bias)^2
    v_quad = a @ v (per chunk)
    kv = sum_{g,c} k_lin^T v                # [S, E] global per batch
    v_lin = q_lin @ kv
    u = tanh(x @ Wue + bue)
    o = (u * (v_quad + v_lin)) @ Wod + bod  # [SRC, D]

Sharding: pure data parallel over batch (B=8) -> one batch element per core.
All matmuls in bf16 with fp32 PSUM accumulation. Host pre-transposes x to
xT [D, SRC] per core so the device never transposes activations; the weights'
natural [in, out] layout is already what the PE wants.

Two passes per core:
  pass 1: zT (kept resident), k_lin (via PE transpose), v (spilled bf16 to
          DRAM scratch), kv accumulated across all chunks in a resident PSUM
          tile.
  pass 2: qk^T -> a^T (rel_bias folds into the ACT as a per-partition bias),
          per e-tile: v_quad^T and v_lin^T accumulate into the SAME psum bank,
          u^T = tanh(.. + bue) (per-partition bias), h = u^T*(vq+vl) on DVE,
          o = h^T-matmuls accumulated over e in PSUM, bod added during the
          PSUM->SBUF copy.
"""

import numpy as np
import ml_dtypes

import concourse.bass as bass
import concourse.tile as tile
from concourse.tile import add_dep_helper
from concourse import bacc, mybir
from concourse.bass_utils import run_bass_kernel_spmd
from concourse.masks import make_identity

BF16 = mybir.dt.bfloat16
F32 = mybir.dt.float32
AF = mybir.ActivationFunctionType

D = 1024      # embed dim
E = 2048      # expanded dim
S = 128       # shrunken attn dim
C = 256       # chunk size
SRC = 4096    # sequence length
G = SRC // C  # 16 chunks
B = 8         # batch == n cores
KD = D // 128   # 8 k-tiles over D
KE = E // 128   # 16 e-tiles


def build_nc() -> bacc.Bacc:
    nc = bacc.Bacc(None, target_bir_lowering=False, debug=False)

    # ---- I/O ----
    xT_h = nc.declare_dram_parameter("xT", [128, KD, SRC], BF16, isOutput=False)
    wxs_h = nc.declare_dram_parameter("Wxs", [128, KD, S], BF16, isOutput=False)
    wve_h = nc.declare_dram_parameter("Wve", [128, KD, E], BF16, isOutput=False)
    wue_h = nc.declare_dram_parameter("Wue", [128, KD, E], BF16, isOutput=False)
    wod_h = nc.declare_dram_parameter("Wod", [128, KE, D], BF16, isOutput=False)
    bxs_h = nc.declare_dram_parameter("bxs", [128, 1], F32, isOutput=False)
    bve_h = nc.declare_dram_parameter("bve", [128, E], BF16, isOutput=False)
    bue_h = nc.declare_dram_parameter("bue", [128, KE], F32, isOutput=False)
    bod_h = nc.declare_dram_parameter("bod", [128, D], F32, isOutput=False)
    rel_h = nc.declare_dram_parameter("rel", [128, 2], F32, isOutput=False)
    gam_h = nc.declare_dram_parameter("gam", [128, 4], F32, isOutput=False)
    bet_h = nc.declare_dram_parameter("bet", [128, 4], F32, isOutput=False)
    o_h = nc.declare_dram_parameter("o", [SRC, D], F32, isOutput=True)

    v_spill = nc.dram_tensor("v_spill", [SRC, E], BF16)

    with tile.TileContext(nc) as tc:
        with (
            tc.tile_pool(name="consts", bufs=1) as consts,
            tc.tile_pool(name="xp", bufs=3) as xp,
            tc.tile_pool(name="vp", bufs=4) as vp,
            tc.tile_pool(name="small", bufs=3) as small,
            tc.tile_pool(name="op", bufs=3) as osb,
        ):
            # ---- resident constants ----
            w_xs = consts.tile([128, KD, S], BF16)
            w_ve = consts.tile([128, KD, E], BF16)
            w_ue = consts.tile([128, KD, E], BF16)
            w_od = consts.tile([128, KE, D], BF16)
            b_xs = consts.tile([128, 1], F32)
            b_ve = consts.tile([128, E], BF16)
            b_ue = consts.tile([128, KE], F32)
            b_od = consts.tile([128, D], F32)
            relb = consts.tile([128, 2], F32)
            gam = consts.tile([128, 4], F32)
            bet = consts.tile([128, 4], F32)
            ident = consts.tile([128, 128], BF16)
            zT_all = consts.tile([128, G, C], BF16)
            kv_sb = consts.tile([128, E], BF16)

            # HAM warmup + DMA-window cover: ~300 dummy matmuls (~18us) keep
            # the PE busy while the ~6MB of first-touch weights stream from
            # HBM. Without this the PE idles in 3-7us holes waiting for Wve
            # slices, re-throttling the clock to 1.2GHz (HAM) and running
            # the first ~30us of real matmuls at half rate.
            with tc.tile_pool(name="pwarm", bufs=1, space="PSUM") as pwarm:
                warm_in = consts.tile([128, 128], BF16)
                nc.vector.memset(warm_in[:], 0.0)
                warm_ps = pwarm.tile([128, 128], F32)
                for _ in range(140):
                    nc.tensor.matmul(warm_ps[:], warm_in[:], warm_in[:],
                                     start=True, stop=True)

            # Head-latency critical path: the sync HWDGE ring carries only
            # what the first chunk needs, in need-order (Wxs -> x chunk 0 ->
            # Wve streamed per k-tile). Small pass-1 consts ride the scalar
            # ring early; the 8.5MB of pass-2-only weights are emitted later
            # (inside the pass-1 loop) so DMA-semaphore recycling never makes
            # a critical load wait on a big transfer.
            nc.sync.dma_start(out=w_xs[:], in_=wxs_h[:])
            nc.scalar.dma_start(out=b_xs[:], in_=bxs_h[:])
            nc.scalar.dma_start(out=gam[:], in_=gam_h[:])
            nc.scalar.dma_start(out=bet[:], in_=bet_h[:])
            nc.scalar.dma_start(out=b_ve[:], in_=bve_h[:])
            make_identity(nc, ident[:])

            # ================= PASS 1 =================
            with (
                tc.tile_pool(name="psz", bufs=2, space="PSUM") as psz,
                tc.tile_pool(name="psv", bufs=2, space="PSUM") as psv,
                tc.tile_pool(name="pkv", bufs=1, space="PSUM") as pkv,
            ):
                kv_ps = pkv.tile([128, E], F32)

                xgs: dict = {}

                def get_xg(g):
                    if g not in xgs:
                        t = xp.tile([128, KD, C], BF16, tag="xg",
                                    name=f"xg{g}")
                        sl = slice(g * C, (g + 1) * C)
                        nc.sync.dma_start(out=t[:], in_=xT_h[:, :, sl])
                        xgs[g] = t
                    return xgs[g]

                def z_front(g):
                    """zT[g] = tanh(Wxs^T x^T + bxs) + k_lin^T affine.
                    Emitted one chunk ahead of its v-block so the PE->ACT
                    round trip hides under the previous chunk's matmuls."""
                    xg = get_xg(g)
                    z_ps = psz.tile([128, C], F32, tag="zps", name=f"zps{g}")
                    for kt in range(KD):
                        nc.tensor.matmul(
                            z_ps[:], w_xs[:, kt, :], xg[:, kt, :],
                            start=(kt == 0), stop=(kt == KD - 1),
                        )
                    nc.scalar.activation(
                        zT_all[:, g, :], z_ps[:], AF.Tanh, bias=b_xs[:, 0:1],
                    )
                    klT = small.tile([128, C], BF16, tag="klT",
                                     name=f"klT{g}")
                    nc.scalar.activation(
                        klT[:], zT_all[:, g, :], AF.Identity,
                        bias=bet[:, 3:4], scale=gam[:, 3:4],
                    )
                    return klT

                def z_back(g, klT):
                    """PE-transpose k_lin^T -> k_lin [C, S]; emitted after the
                    previous chunk's matmul block so the PE never waits on the
                    ACT affine."""
                    kl = small.tile([128, 2, S], BF16, tag="kl",
                                    name=f"kl{g}")
                    for ct in range(2):
                        t_ps = psz.tile([128, 128], BF16, tag="zps",
                                        name=f"tps{g}_{ct}")
                        nc.tensor.transpose(
                            t_ps[:], klT[:, ct * 128:(ct + 1) * 128], ident[:]
                        )
                        nc.vector.tensor_copy(kl[:, ct, :], t_ps[:])
                    return kl

                get_xg(0)
                # ns-major streaming matches the v-loop's consumption order:
                # the (ct0, ns0) psum group needs all k-tiles of e-cols 0:512.
                # xg1 is prefetched after the first slice so chunk 1's zT
                # isn't stuck behind the remaining 3MB of Wve.
                for ns in range(E // 512):
                    esl = slice(ns * 512, (ns + 1) * 512)
                    nc.sync.dma_start(out=w_ve[:, :, esl], in_=wve_h[:, :, esl])
                    if ns == 0:
                        get_xg(1)
                kls = {0: z_back(0, z_front(0))}
                klTs: dict = {}
                for g in range(G):
                    xg = get_xg(g)
                    if g + 1 < G:
                        klTs[g + 1] = z_front(g + 1)
                    kl = kls.pop(g)

                    # v = tanh(x @ Wve + bve), spill to DRAM; kv += k_lin^T v
                    for ct in range(2):
                        vt = vp.tile([128, E], BF16, tag="v")
                        for ns in range(E // 512):
                            v_ps = psv.tile([128, 512], F32, tag="vps")
                            esl = slice(ns * 512, (ns + 1) * 512)
                            for kt in range(KD):
                                nc.tensor.matmul(
                                    v_ps[:], xg[:, kt, ct * 128:(ct + 1) * 128],
                                    w_ve[:, kt, esl],
                                    start=(kt == 0), stop=(kt == KD - 1),
                                )
                            nc.vector.tensor_add(v_ps[:], v_ps[:], b_ve[:, esl])
                            nc.scalar.activation(vt[:, esl], v_ps[:], AF.Tanh)
                        r0 = g * C + ct * 128
                        nc.sync.dma_start(out=v_spill[r0:r0 + 128, :], in_=vt[:])
                        for ns in range(E // 512):
                            esl = slice(ns * 512, (ns + 1) * 512)
                            mm = nc.tensor.matmul(
                                kv_ps[:, esl], kl[:, ct, :], vt[:, esl],
                                start=(g == 0 and ct == 0),
                                stop=(g == G - 1 and ct == 1),
                            )
                            if ct == 0 and ns == 0:
                                kv_gate = mm
                    # Pass-2-only weights trickled in ~1MB slices, each
                    # gated behind this chunk's kv matmul. Without the dep
                    # the scheduler hoists the no-dependency DMAs to t~10us
                    # where 9MB halves Wve's bandwidth (first v-wave stalls
                    # to ~37us); one big gated burst instead starves the
                    # xg3 load behind 8.5MB of FIFO (5.6us hole at ~56us).
                    es2 = E // 4
                    if g < 4:        # w_ue in 4 column slices
                        di = nc.sync.dma_start(
                            out=w_ue[:, :, g * es2:(g + 1) * es2],
                            in_=wue_h[:, :, g * es2:(g + 1) * es2])
                    elif g < 8:      # w_od in 4 e-tile-row slices
                        j = g - 4
                        di = nc.sync.dma_start(
                            out=w_od[:, j * 4:(j + 1) * 4, :],
                            in_=wod_h[:, j * 4:(j + 1) * 4, :])
                    elif g == 8:
                        for dst, src in ((b_ue, bue_h), (b_od, bod_h),
                                         (relb, rel_h)):
                            di = nc.sync.dma_start(out=dst[:], in_=src[:])
                            add_dep_helper(di.ins, kv_gate.ins, sync=True,
                                           reason="defer pass-2 consts")
                        di = None
                    else:
                        di = None
                    if di is not None:
                        add_dep_helper(di.ins, kv_gate.ins, sync=True,
                                       reason="defer pass-2 weights")
                    if g + 1 < G:
                        kls[g + 1] = z_back(g + 1, klTs.pop(g + 1))
                    del xgs[g]

                for ns in range(E // 512):
                    esl = slice(ns * 512, (ns + 1) * 512)
                    nc.vector.tensor_copy(kv_sb[:, esl], kv_ps[:, esl])

            # ================= PASS 2 =================
            with (
                tc.tile_pool(name="ps2", bufs=4, space="PSUM") as ps2,
                tc.tile_pool(name="po", bufs=2, space="PSUM") as po,
            ):
                for g in range(G):
                    xg = xp.tile([128, KD, C], BF16, tag="xg")
                    nc.sync.dma_start(out=xg[:], in_=xT_h[:, :, g * C:(g + 1) * C])
                    vts = []
                    for ct in range(2):
                        vt = vp.tile([128, E], BF16, tag="v")
                        r0 = g * C + ct * 128
                        nc.sync.dma_start(out=vt[:], in_=v_spill[r0:r0 + 128, :])
                        vts.append(vt)

                    # affines of z (all in [S, C] layout, per-partition consts)
                    qqT = small.tile([128, C], BF16, tag="qqT")
                    kqT = small.tile([128, C], BF16, tag="kqT")
                    qlT = small.tile([128, C], BF16, tag="qlT")
                    zg = zT_all[:, g, :]
                    nc.scalar.activation(qqT[:], zg, AF.Identity,
                                         bias=bet[:, 0:1], scale=gam[:, 0:1])
                    nc.scalar.activation(kqT[:], zg, AF.Identity,
                                         bias=bet[:, 1:2], scale=gam[:, 1:2])
                    nc.scalar.activation(qlT[:], zg, AF.Identity,
                                         bias=bet[:, 2:3], scale=gam[:, 2:3])

                    # a^T[m, n] = relu(qk^T[m, n] + rel_bias[m])^2
                    aT = small.tile([128, 2, C], BF16, tag="aT")
                    for mt in range(2):
                        qk_ps = ps2.tile([128, C], F32, tag="ps2")
                        nc.tensor.matmul(
                            qk_ps[:], kqT[:, mt * 128:(mt + 1) * 128], qqT[:],
                            start=True, stop=True,
                        )
                        rl = small.tile([128, C], BF16, tag="rl")
                        nc.scalar.activation(rl[:], qk_ps[:], AF.Relu,
                                             bias=relb[:, mt:mt + 1])
                        nc.vector.tensor_mul(aT[:, mt, :], rl[:], rl[:])

                    # per e-tile: (v_quad + v_lin)^T, u^T, h, o-accumulation
                    o_ps = [po.tile([128, D], F32, tag="o", name=f"o_ps{i}")
                            for i in range(2)]
                    for et in range(KE):
                        q_ps = ps2.tile([128, C], F32, tag="ps2")
                        etsl = slice(et * 128, (et + 1) * 128)
                        for mt in range(2):
                            nc.tensor.matmul(
                                q_ps[:], vts[mt][:, etsl], aT[:, mt, :],
                                start=(mt == 0), stop=False,
                            )
                        nc.tensor.matmul(
                            q_ps[:], kv_sb[:, etsl], qlT[:],
                            start=False, stop=True,
                        )
                        u_ps = ps2.tile([128, C], F32, tag="ps2")
                        for kt in range(KD):
                            nc.tensor.matmul(
                                u_ps[:], w_ue[:, kt, etsl], xg[:, kt, :],
                                start=(kt == 0), stop=(kt == KD - 1),
                            )
                        ut = small.tile([128, C], BF16, tag="ut")
                        nc.scalar.activation(ut[:], u_ps[:], AF.Tanh,
                                             bias=b_ue[:, et:et + 1])
                        ht = small.tile([128, C], BF16, tag="ht")
                        nc.vector.tensor_mul(ht[:], ut[:], q_ps[:])
                        for ct in range(2):
                            csl = slice(ct * 128, (ct + 1) * 128)
                            for ds in range(2):
                                dsl = slice(ds * 512, (ds + 1) * 512)
                                nc.tensor.matmul(
                                    o_ps[ct][:, dsl], ht[:, csl], w_od[:, et, dsl],
                                    start=(et == 0), stop=(et == KE - 1),
                                )

                    for ct in range(2):
                        ot = osb.tile([128, D], F32, tag="o_sb")
                        nc.vector.tensor_add(ot[:], o_ps[ct][:], b_od[:])
                        r0 = g * C + ct * 128
                        nc.sync.dma_start(out=o_h[r0:r0 + 128, :], in_=ot[:])

    return nc


_CACHE: dict = {}


def _prep_inputs(value, Wxs, bxs, Wve, bve, Wue, bue, Wod, bod,
                 rel_bias, gamma, beta):
    bf = ml_dtypes.bfloat16
    shared = {
        "Wxs": np.ascontiguousarray(
            Wxs.astype(bf).reshape(KD, 128, S).transpose(1, 0, 2)),
        "Wve": np.ascontiguousarray(
            Wve.astype(bf).reshape(KD, 128, E).transpose(1, 0, 2)),
        "Wue": np.ascontiguousarray(
            Wue.astype(bf).reshape(KD, 128, E).transpose(1, 0, 2)),
        "Wod": np.ascontiguousarray(
            Wod.astype(bf).reshape(KE, 128, D).transpose(1, 0, 2)),
        "bxs": np.ascontiguousarray(bxs.astype(np.float32).reshape(128, 1)),
        "bve": np.ascontiguousarray(
            np.broadcast_to(bve.astype(bf), (128, E))),
        "bue": np.ascontiguousarray(
            bue.astype(np.float32).reshape(KE, 128).T),
        "bod": np.ascontiguousarray(
            np.broadcast_to(bod.astype(np.float32), (128, D))),
        "rel": np.ascontiguousarray(
            rel_bias.astype(np.float32).reshape(2, 128).T),
        "gam": np.ascontiguousarray(gamma.astype(np.float32).T),
        "bet": np.ascontiguousarray(beta.astype(np.float32).T),
    }
    in_maps = []
    for b in range(B):
        xT = np.ascontiguousarray(value[:, b, :].T.astype(bf)
                                  .reshape(KD, 128, SRC).transpose(1, 0, 2))
        in_maps.append({"xT": xT, **shared})
    return in_maps


def kernel(**inputs) -> np.ndarray:
    inp = {k: np.asarray(v) for k, v in inputs.items()}
    in_maps = _prep_inputs(
        inp["value"], inp["Wxs"], inp["bxs"], inp["Wve"], inp["bve"],
        inp["Wue"], inp["bue"], inp["Wod"], inp["bod"],
        inp["rel_bias"], inp["gamma"], inp["beta"],
    )
    if "nc" not in _CACHE:
        nc = build_nc()
        nc.compile()
        _CACHE["nc"] = nc
    res = run_bass_kernel_spmd(_CACHE["nc"], in_maps, list(range(B))).results
    out = np.stack([r["o"] for r in res], axis=1)
    return np.ascontiguousarray(out.astype(np.float32))
